# revision 84
# baseline (speedup 1.0000x reference)
"""MultiHeadAttention (B=4,S=2048,E=512,H=8) on 8 Trainium2 cores.

Sharding: core c -> (batch b = c//2, head-group hg = c%2, 4 heads each).
Each core computes its 4 heads' attention + its slice of out_proj rows;
host sums the two partial outputs per batch.

Structure (final):
  - Token-major projections ([128 tok, 256 feat] tiles, evac/square/
    reduce batched per QUAD of tiles) so the per-head LayerNorm is a
    per-token tensor_scalar: no cross-partition rstd broadcast, no DRAM
    roundtrip.  LN centering is folded into the projection weights on
    host; rstd = 1/sqrt(ss/64+eps) via ACT Sqrt (single table set; Exp
    loads once for the whole attention phase) + DVE reciprocal,
    computed per 8-tile (K) / 4-tile (Q, V) group.
  - K/Q projections run as fp8e4m3 DoubleRow matmuls: inputs and
    weights (x16 so fp8 stays in normal range - LN absorbs any uniform
    scale; the 256x score factor is folded into the rstd and exp
    scales) ship packed [kfpair, 128, 2, *].  V stays f16 end to end.
  - Normalized K/Q are PE-transposed into packed fp8 kh8/qh8 per
    head-pair [64(2h x 32), 2, S] (PE base partitions limited to
    0/32/64), so scores run as fp8 DoubleRow matmuls at 0.5 cycles/row
    - 4x fewer PE cycles than the f16 K=64 form.
  - exp applied PSUM->SBUF f16 by ACT with a constant scale (no
    max-subtraction; logits are tiny).  Softmax denominator and key
    mask ride in an augmented V row (vh_aug[...,64] = km).  The oa PSUM
    accumulators are freed by an immediate outT copy; the
    1/denominator multiply is deferred past the DRAM-broadcast
    roundtrip (PE K=1 broadcast matmul on the last q-chunk).
  - Attention loops q-chunks outer, head-pair + key-tile inner, with
    the next block's first scores+exp pre-issued before the last oa so
    ACT stays saturated across block boundaries.  V augmentation, Q
    transpose/pack for later q-chunks, and out_proj for the previous
    q-chunk drain from a backlog between iterations, hiding under the
    ACT-bound exp pipeline.
  - query mask applied once on the final (f16) output; value mask
    folded into the aug step as a host-premultiplied km*vm column
    (masked rows zero out regardless of their rstd); host accumulates
    the two per-batch partial outputs in f32.
"""

import numpy as np
from collections import deque
from contextlib import ExitStack

from concourse import bacc
import concourse.mybir as mybir
import concourse.tile as tile
from concourse.masks import make_identity
from concourse.bass_utils import run_bass_kernel_spmd

B, S, E, H = 4, 2048, 512, 8
D = E // H                     # 64
SCALE = float(E // H ** 0.5)   # 181.0 (faithful to the torch precedence bug)
EPS = 1e-5
HL = H // 2                    # heads per core = 4
OF = HL * D                    # local out-features = 256
FT = E // 128                  # 4 input-feature tiles

f16 = mybir.dt.float16
f32 = mybir.dt.float32
f8 = mybir.dt.float8e4

_prog_cache = {}


def build_program(S_=S, flags=(False,) * 10):
    (bq_nz, bk_nz, bv_nz, bo_nz, betaq_nz, betak_nz, betav_nz,
     gq_ne1, gk_ne1, gv_ne1) = flags

    NT = S_ // 128            # token tiles
    CH = min(512, S_)         # q-chunk width
    NQC = S_ // CH            # q chunks
    TPC = CH // 128           # token tiles per q-chunk
    # prologue covers these; the rest chase inside the attention loop
    NVP = min(8, NT)          # V tiles prepped in prologue
    NQP = min(TPC, NT)        # Q tiles prepped in prologue
    Exp = mybir.ActivationFunctionType.Exp
    Sqrt = mybir.ActivationFunctionType.Sqrt
    DR = mybir.MatmulPerfMode.DoubleRow
    # fp8 DoubleRow projections for K/Q when no bias/beta/g adjustments:
    # inputs + weights(x16, LN absorbs the scale) packed [*, 2, *] fp8
    fp8_kq = not (bq_nz or bk_nz or betaq_nz or betak_nz or gq_ne1 or gk_ne1)

    nc = bacc.Bacc()

    if fp8_kq:
        qT8 = nc.declare_dram_parameter("qT8", [2, 128, 2, S_], f8,
                                        isOutput=False)
        kT8 = nc.declare_dram_parameter("kT8", [2, 128, 2, S_], f8,
                                        isOutput=False)
        wq8 = nc.declare_dram_parameter("wq8", [2, 128, 2, OF], f8,
                                        isOutput=False)
        wk8 = nc.declare_dram_parameter("wk8", [2, 128, 2, OF], f8,
                                        isOutput=False)
    else:
        qT = nc.declare_dram_parameter("qT", [E, S_], f16, isOutput=False)
        kT = nc.declare_dram_parameter("kT", [E, S_], f16, isOutput=False)
        wqT = nc.declare_dram_parameter("wqT", [E, OF], f16, isOutput=False)
        wkT = nc.declare_dram_parameter("wkT", [E, OF], f16, isOutput=False)
    vT = nc.declare_dram_parameter("vT", [E, S_], f16, isOutput=False)
    wvT = nc.declare_dram_parameter("wvT", [E, OF], f16, isOutput=False)
    woT = nc.declare_dram_parameter("woT", [OF, E], f16, isOutput=False)
    qm = nc.declare_dram_parameter("qm", [128, NT], f32, isOutput=False)
    km = nc.declare_dram_parameter("km", [128, NT], f32, isOutput=False)
    vm = nc.declare_dram_parameter("vm", [128, NT], f32, isOutput=False)
    kvm = nc.declare_dram_parameter("kvm", [128, NT], f32, isOutput=False)
    cond = {}
    if bq_nz:
        cond["bqc"] = nc.declare_dram_parameter("bqc", [1, OF], f16, isOutput=False)
    if bk_nz:
        cond["bkc"] = nc.declare_dram_parameter("bkc", [1, OF], f16, isOutput=False)
    if bv_nz:
        cond["bvc"] = nc.declare_dram_parameter("bvc", [1, OF], f16, isOutput=False)
    if bo_nz:
        cond["bo2"] = nc.declare_dram_parameter("bo2", [1, E], f16, isOutput=False)
    for nm, flag in (("betaq", betaq_nz), ("betak", betak_nz),
                     ("betav", betav_nz), ("gqinv2", gq_ne1),
                     ("gkinv2", gk_ne1), ("gvinv2", gv_ne1)):
        if flag:
            cond[nm] = nc.declare_dram_parameter(nm + "_r", [1, OF], f32,
                                                 isOutput=False)
    out = nc.declare_dram_parameter("out", [S_, E], f16, isOutput=True)
    denom_dram = nc.dram_tensor("denom_dram", [HL, S_], f32)

    with tile.TileContext(nc) as tc, ExitStack() as ctx:
        const = ctx.enter_context(tc.tile_pool(name="const", bufs=1))
        persist = ctx.enter_context(tc.tile_pool(name="persist", bufs=1))
        inpool = ctx.enter_context(tc.tile_pool(name="inpool", bufs=3))
        xpool = ctx.enter_context(tc.tile_pool(name="xpool", bufs=3))
        xn_pool = ctx.enter_context(tc.tile_pool(name="xn", bufs=3))
        sq_pool = ctx.enter_context(tc.tile_pool(name="sq", bufs=3))
        pT_pool = ctx.enter_context(tc.tile_pool(name="pT", bufs=4))
        dn_pool = ctx.enter_context(tc.tile_pool(name="dn", bufs=2))
        rsb_pool = ctx.enter_context(tc.tile_pool(name="rsb", bufs=2))
        fin_pool = ctx.enter_context(tc.tile_pool(name="fin", bufs=2))

        # ---- constants ----
        ident = const.tile([128, 128], f16, tag="ident", name="ident")
        make_identity(nc, ident)
        eps_col = const.tile([128, 1], f32, tag="eps_col", name="eps_col")
        nc.vector.memset(eps_col, EPS)
        ones_row = const.tile([1, 128], f16, tag="ones_row", name="ones_row")
        nc.vector.memset(ones_row, 1.0)

        wk_sb = const.tile([128, FT, OF], f16, tag="wk", name="wk")
        wv_sb = const.tile([128, FT, OF], f16, tag="wv", name="wv")
        wq_sb = const.tile([128, FT, OF], f16, tag="wq", name="wq")
        wo_sb = const.tile([64, HL, E], f16, tag="wo", name="wo")
        qm_sb = const.tile([128, NT], f32, tag="qm", name="qm")
        km_sb = const.tile([128, NT], f32, tag="km", name="km")
        vm_sb = const.tile([128, NT], f32, tag="vm", name="vm")
        kvm_sb = const.tile([128, NT], f32, tag="kvm", name="kvm")

        # input DMAs, all on the sync queue in consumption order
        # (scalar queue stays clear so ACT evacs aren't blocked);
        # masks on gpsimd.
        in_v = inpool.tile([128, FT, S_], f16, tag="in", name="in_v")
        HC = max(S_ // 2, 128)
        NHC = S_ // HC
        if fp8_kq:
            in_k = inpool.tile([128, 2, 2, S_], f8, tag="in8", name="in_k")
            in_q = inpool.tile([128, 2, 2, S_], f8, tag="in8", name="in_q")
            wk8_sb = const.tile([128, 2, 2, OF], f8, tag="wk8", name="wk8")
            wq8_sb = const.tile([128, 2, 2, OF], f8, tag="wq8", name="wq8")
            nc.sync.dma_start(out=wk8_sb, in_=wk8.rearrange("f p i m -> p f i m"))
            for hc in range(NHC):
                for kfp in range(2):
                    nc.sync.dma_start(
                        out=in_k[:, kfp, :, hc * HC:(hc + 1) * HC],
                        in_=kT8[kfp, :, :, hc * HC:(hc + 1) * HC])
        else:
            in_k = inpool.tile([128, FT, S_], f16, tag="in", name="in_k")
            in_q = inpool.tile([128, FT, S_], f16, tag="in", name="in_q")
            nc.sync.dma_start(out=wk_sb,
                              in_=wkT.rearrange("(t p) m -> p t m", p=128))
            for hc in range(NHC):
                for kf in range(FT):
                    nc.sync.dma_start(
                        out=in_k[:, kf, hc * HC:(hc + 1) * HC],
                        in_=kT[kf * 128:(kf + 1) * 128, hc * HC:(hc + 1) * HC])
        nc.sync.dma_start(out=km_sb, in_=km[:])
        nc.sync.dma_start(out=vm_sb, in_=vm[:])
        nc.sync.dma_start(out=kvm_sb, in_=kvm[:])
        nc.sync.dma_start(out=qm_sb, in_=qm[:])
        if fp8_kq:
            nc.sync.dma_start(out=wq8_sb, in_=wq8.rearrange("f p i m -> p f i m"))
            for hc in range(NHC):
                for kfp in range(2):
                    nc.sync.dma_start(
                        out=in_q[:, kfp, :, hc * HC:(hc + 1) * HC],
                        in_=qT8[kfp, :, :, hc * HC:(hc + 1) * HC])
        else:
            nc.sync.dma_start(out=wq_sb,
                              in_=wqT.rearrange("(t p) m -> p t m", p=128))
            for hc in range(NHC):
                for kf in range(FT):
                    nc.sync.dma_start(
                        out=in_q[:, kf, hc * HC:(hc + 1) * HC],
                        in_=qT[kf * 128:(kf + 1) * 128, hc * HC:(hc + 1) * HC])
        nc.sync.dma_start(out=wv_sb, in_=wvT.rearrange("(t p) m -> p t m", p=128))
        for hc in range(NHC):
            for kf in range(FT):
                nc.sync.dma_start(
                    out=in_v[:, kf, hc * HC:(hc + 1) * HC],
                    in_=vT[kf * 128:(kf + 1) * 128, hc * HC:(hc + 1) * HC])
        for hh in range(HL):
            nc.sync.dma_start(out=wo_sb[:, hh, :],
                              in_=woT[hh * 64:(hh + 1) * 64, :])
        brow = {}
        for nm, flag in (("bkc", bk_nz), ("bvc", bv_nz), ("bqc", bq_nz)):
            if flag:
                brow[nm] = const.tile([1, OF], f16, tag=nm, name=nm)
                nc.gpsimd.dma_start(out=brow[nm], in_=cond[nm][:])
        if bo_nz:
            bo2_sb = const.tile([1, E], f16, tag="bo2", name="bo2")
            nc.gpsimd.dma_start(out=bo2_sb, in_=cond["bo2"][:])
        rep = {}
        for nm in ("betaq", "betak", "betav", "gqinv2", "gkinv2", "gvinv2"):
            if nm in cond:
                rep[nm] = const.tile([128, OF], f32, tag=nm, name=nm)
                nc.gpsimd.dma_start(out=rep[nm],
                                    in_=cond[nm][:].to_broadcast((128, OF)))

        # ---- persistent tensors ----
        # packed fp8 per head-pair hp: head 2hp at partitions 0:32,
        # head 2hp+1 at 32:64 (PE base-partition must be 0/32/64)
        kh8 = [persist.tile([64, 2, S_], f8, tag=f"kh8_{hp}", name=f"kh8_{hp}")
               for hp in range(2)]
        qh8 = [persist.tile([64, 2, S_], f8, tag=f"qh8_{hp}", name=f"qh8_{hp}")
               for hp in range(2)]
        vh_aug = persist.tile([128, NT, HL, 65], f16, tag="vh_aug", name="vh_aug")
        outT = [persist.tile([64, S_], f16, tag=f"oT{h}", name=f"oT{h}")
                for h in range(HL)]
        stats = {nm: persist.tile([128, NT, HL], f32, tag=f"st_{nm}",
                                  name=f"st_{nm}")
                 for nm in ("k", "v", "q")}
        xp = {nm: xpool.tile([128, NT, OF], f16, tag="xp", name=f"xp_{nm}")
              for nm in ("k", "v", "q")}

        if fp8_kq:
            TENSOR_CFG = {
                "k": (in_k, wk8_sb, None, None),
                "q": (in_q, wq8_sb, None, None),
            }
        else:
            TENSOR_CFG = {
                "k": (in_k, wk_sb, "bkc" if bk_nz else None,
                      "gkinv2" if gk_ne1 else None),
                "q": (in_q, wq_sb, "bqc" if bq_nz else None,
                      "gqinv2" if gq_ne1 else None),
            }
        TENSOR_CFG["v"] = (in_v, wv_sb, "bvc" if bv_nz else None,
                           "gvinv2" if gv_ne1 else None)

        def proj_tile(name, pp_pool, t, evac_act, pp_bufs=3,
                      pp_tag="pp", pair=None):
            """PE: project token tile t.  With pair=(ps, slot), projections
            accumulate into a shared [128, 2, OF] PSUM tile and evac/sq/
            reduce run once per pair (halves instr count)."""
            in_g, w_sb, bnm, gnm = TENSOR_CFG[name]
            if pair is None:
                ps_full = pp_pool.tile([128, 2, OF], f32, tag=pp_tag,
                                       name="pp", bufs=pp_bufs)
                slots = ((t, 0),)
                do_post = True
            else:
                ps_full, slot, do_post = pair
                slots = ((t, slot),)
            for tt, sl in slots:
                ps = ps_full[:, sl, :]
                if fp8_kq and name != "v":
                    for kfp in range(2):
                        nc.tensor.matmul(
                            ps,
                            lhsT=in_g[:, kfp, :, tt * 128:(tt + 1) * 128],
                            rhs=w_sb[:, kfp, :, :],
                            start=(kfp == 0), stop=(kfp == 1),
                            perf_mode=DR)
                else:
                    for kf in range(FT):
                        nc.tensor.matmul(
                            ps,
                            lhsT=in_g[:, kf, tt * 128:(tt + 1) * 128],
                            rhs=w_sb[:, kf, :],
                            start=(kf == 0),
                            stop=(kf == FT - 1 and bnm is None))
                    if bnm is not None:
                        nc.tensor.matmul(ps, lhsT=ones_row, rhs=brow[bnm],
                                         start=False, stop=True)
            if not do_post:
                return
            # post: evac + square + reduce over the tile(s) in ps_full
            nb = 1 if pair is None else ps_full.shape[1]
            t0 = t - (nb - 1)
            n = nb
            src = ps_full[:, 0:n, :]
            dst = xp[name][:, t0:t0 + n, :]
            if evac_act:
                nc.scalar.copy(dst, src)
            else:
                nc.vector.tensor_copy(out=dst, in_=src)
            sq = sq_pool.tile([128, 4, OF], f16, tag="sq", name="sq")
            sqv = sq[:, 0:n, :]
            nc.vector.tensor_tensor(sqv, dst, dst, mybir.AluOpType.mult)
            if gnm is not None:
                nc.vector.tensor_tensor(
                    sqv, sqv,
                    rep[gnm][:].to_broadcast((128, n, OF))
                    if n == 2 else rep[gnm],
                    mybir.AluOpType.mult)
            nc.vector.tensor_reduce(
                out=stats[name][:, t0:t0 + n, :],
                in_=sqv.rearrange("p n (h d) -> p n h d", d=D),
                axis=mybir.AxisListType.X,
                op=mybir.AluOpType.add)

        def proj_pair(name, pp_pool, t0, evac_act, nq=4):
            """Project tiles t0..t0+nq-1 sharing one quad PSUM tile."""
            ps_full = pp_pool.tile([128, nq, OF], f32, tag="pp", name="pp",
                                   bufs=2)
            for j in range(nq):
                proj_tile(name, pp_pool, t0 + j, evac_act,
                          pair=(ps_full, j, j == nq - 1))

        def rstd_block(name, t0, t1):
            """rstd = 1/sqrt(sumsq/D+eps) in-place on stats: ACT Sqrt
            (stays in the sqrt table set; exp loads once at attention) +
            DVE reciprocal.
            fp8 K/Q carry a x16 weight scale -> sumsq is 256x the true
            sum of squares; fold 1/256 into the Ln scale so rstd comes out
            in true units (kh/qh stay 16x, absorbed by the exp scale)."""
            sc_ = 1.0 / D / (256.0 if fp8_kq and name != "v" else 1.0)
            blk = stats[name][:, t0:t1, :]
            nc.scalar.activation(blk, blk, Sqrt, bias=eps_col, scale=sc_)
            nc.vector.reciprocal(blk, blk)

        def norm_transp(name, t, psT, half, on_pool=False):
            """Pool/DVE: xn = xp*rstd (+beta); PE: transpose into packed
            psT[64, hp, i, 256] (head 2hp+j at partition base 32j)."""
            eng = nc.gpsimd if on_pool else nc.vector
            xn = xn_pool.tile([128, OF], f16, tag="xn", name="xn")
            bnm = "beta" + name
            for h in range(HL):
                eng.tensor_scalar_mul(
                    out=xn[:, h * D:(h + 1) * D],
                    in0=xp[name][:, t, h * D:(h + 1) * D],
                    scalar1=stats[name][:, t, h:h + 1])
            if bnm in rep:
                eng.tensor_tensor(xn, xn, rep[bnm], mybir.AluOpType.add)
            for h in range(HL):
                base = 32 * (h % 2)
                for i in range(2):
                    nc.tensor.transpose(
                        psT[base:base + 32, h // 2, i,
                            half * 128:(half + 1) * 128],
                        xn[:, h * D + 32 * i:h * D + 32 * (i + 1)],
                        ident)

        def pack_evac(dst8, tp, psT, mode):
            """mode: 'split' (ACT hp0 / DVE hp1), 'dve' (both on DVE)."""
            for hp in range(2):
                sl = dst8[hp][:, :, tp * 256:(tp + 1) * 256]
                if mode == "act" or (mode == "split" and hp == 0):
                    nc.scalar.copy(sl, psT[:, hp])
                else:
                    nc.vector.tensor_copy(out=sl, in_=psT[:, hp])

        def aug_tile(t):
            """Pool: vh_aug[:, t] = [vp*rstd*km | km]."""
            for h in range(HL):
                nc.gpsimd.tensor_scalar(
                    out=vh_aug[:, t, h, 0:D],
                    in0=xp["v"][:, t, h * D:(h + 1) * D],
                    scalar1=stats["v"][:, t, h:h + 1],
                    scalar2=kvm_sb[:, t:t + 1],
                    op0=mybir.AluOpType.mult,
                    op1=mybir.AluOpType.mult)
                if "betav" in rep:
                    tmp = sq_pool.tile([128, D], f32, tag="bvkm", name="bvkm")
                    nc.gpsimd.tensor_scalar_mul(
                        tmp, rep["betav"][:, h * D:(h + 1) * D],
                        km_sb[:, t:t + 1])
                    nc.gpsimd.tensor_tensor(
                        vh_aug[:, t, h, 0:D], vh_aug[:, t, h, 0:D], tmp,
                        mybir.AluOpType.add)
            nc.gpsimd.tensor_copy(
                out=vh_aug[:, t, :, 64:65],
                in_=km_sb[:, t:t + 1].to_broadcast((128, HL, 1)))

        # ====== prologue: ALL projections + stats (batched rstds keep
        # Ln/Exp table switches out of the exp-bound attention phase) ======
        with ExitStack() as pctx:
            pp_pool = pctx.enter_context(
                tc.tile_pool(name="pp", bufs=3, space="PSUM"))
            psT_pool = pctx.enter_context(
                tc.tile_pool(name="psT", bufs=2, space="PSUM"))

            # K -> Q -> V, per-group-of-4 rstds (Sqrt stays in one ACT
            # table set) so each stage's norm/transpose/aug pipelines right
            # behind its projections instead of waiting on a full-tensor
            # barrier.
            G = 4
            KG = 8
            for g in range(NT // KG):
                if fp8_kq:
                    for tp0 in range(g * KG, (g + 1) * KG, 4):
                        proj_pair("k", pp_pool, tp0, evac_act=True)
                else:
                    for t in range(g * KG, (g + 1) * KG):
                        proj_tile("k", pp_pool, t, evac_act=True)
                rstd_block("k", g * KG, (g + 1) * KG)
                for tp in range(g * KG // 2, (g + 1) * KG // 2):
                    psT = psT_pool.tile([64, 2, 2, 256], f16, tag="psT",
                                        name="psT")
                    for half in range(2):
                        norm_transp("k", 2 * tp + half, psT, half,
                                    on_pool=True)
                    pack_evac(kh8, tp, psT, mode="split")

            if fp8_kq:
                for tp0 in range(0, NQP, 4):
                    proj_pair("q", pp_pool, tp0, evac_act=True)
            else:
                for t in range(NQP):
                    proj_tile("q", pp_pool, t, evac_act=True)
            rstd_block("q", 0, NQP)
            for tp in range(NQP // 2):
                psT = psT_pool.tile([64, 2, 2, 256], f16, tag="psT",
                                    name="psT")
                for half in range(2):
                    norm_transp("q", 2 * tp + half, psT, half,
                                on_pool=True)
                pack_evac(qh8, tp, psT, mode="act")
            for g in range(NQP // G, NT // G):
                if fp8_kq:
                    for tp0 in range(g * G, (g + 1) * G, 4):
                        proj_pair("q", pp_pool, tp0, evac_act=True)
                else:
                    for t in range(g * G, (g + 1) * G):
                        proj_tile("q", pp_pool, t, evac_act=(t % 2 == 0))
                rstd_block("q", g * G, (g + 1) * G)
            for g in range(NT // G):
                for tp0 in range(g * G, (g + 1) * G, 4):
                    proj_pair("v", pp_pool, tp0, evac_act=True)
                rstd_block("v", g * G, (g + 1) * G)
                if g * G < NVP:
                    for t in range(g * G, min((g + 1) * G, NVP)):
                        aug_tile(t)


        # ================= attention =================
        with ExitStack() as pctx:
            aux_pool = pctx.enter_context(
                tc.tile_pool(name="aux", bufs=2, space="PSUM"))
            oa_pool = pctx.enter_context(
                tc.tile_pool(name="oa", bufs=2, space="PSUM"))
            sc_pool = pctx.enter_context(
                tc.tile_pool(name="sc", bufs=2, space="PSUM"))

            backlog = deque()

            def drain(n):
                for _ in range(min(n, len(backlog))):
                    backlog.popleft()()

            # V chase: tiles NVP..NT-1, rstd+aug per group of 4
            # V chase: aug only (stats/rstd all done in prologue)
            backlog.extend(lambda t=t: aug_tile(t) for t in range(NVP, NT))

            # Q chase: norm+transp per tile, pack per pair (no ACT work).
            # psT is shared state across the pair's two halves.
            qpair_psT = {}

            def mk_qtransp(tp, half):
                def f():
                    if half == 0:
                        qpair_psT[tp] = aux_pool.tile(
                            [64, 2, 2, 256], f16, tag="psT", name="psT",
                            bufs=1)
                    norm_transp("q", 2 * tp + half, qpair_psT[tp], half)
                    if half == 1:
                        pack_evac(qh8, tp, qpair_psT.pop(tp), mode="dve")
                return f

            def mk_qpair(tp):
                return [mk_qtransp(tp, 0), mk_qtransp(tp, 1)]

            def mk_outproj(t, on_act=False):
                def f():
                    # two half-E accumulations: psf fits a 1KB PSUM buffer
                    fin = fin_pool.tile([128, E], f16, tag="fin", name="fin")
                    for eh in range(2):
                        ecols = slice(eh * (E // 2), (eh + 1) * (E // 2))
                        psf = aux_pool.tile([128, E // 2], f32,
                                            tag="scratch", name="psf",
                                            bufs=1)
                        for h in range(HL):
                            nc.tensor.matmul(
                                psf,
                                lhsT=outT[h][:, t * 128:(t + 1) * 128],
                                rhs=wo_sb[:, h, ecols],
                                start=(h == 0),
                                stop=(h == HL - 1 and not bo_nz))
                        if bo_nz:
                            nc.tensor.matmul(psf, lhsT=ones_row,
                                             rhs=bo2_sb[:, ecols],
                                             start=False, stop=True)
                        if on_act:
                            nc.scalar.mul(fin[:, ecols], psf,
                                          qm_sb[:, t:t + 1])
                        else:
                            nc.vector.tensor_scalar_mul(
                                out=fin[:, ecols], in0=psf,
                                scalar1=qm_sb[:, t:t + 1])
                    nc.sync.dma_start(
                        out=out[t * 128:(t + 1) * 128, :], in_=fin)
                return f

            # kh/qh are 16x each in the fp8 path -> raw scores 256x
            exp_scale = 1.0 / SCALE / (256.0 if fp8_kq else 1.0)

            # normalize-by-denominator mults are deferred to the next block
            # so the oa PSUM tiles are freed by the outT copy immediately
            # (the DRAM-broadcast roundtrip no longer blocks the next hp)
            pending_mults = []

            def flush_mults():
                for f in pending_mults:
                    f()
                pending_mults.clear()

            def mk_mult(h, qcols, rsbd):
                return lambda: nc.vector.tensor_tensor(
                    outT[h][:, qcols], outT[h][:, qcols], rsbd,
                    mybir.AluOpType.mult)

            block_list = [(qc, hp) for qc in range(NQC) for hp in range(2)]
            carried = None  # pre-issued pT for the next block's kt=0

            def scores_for(qc, hp, kt):
                qcols = slice(qc * CH, (qc + 1) * CH)
                sc = sc_pool.tile([128, 2 * CH], f32, tag="sc", name="sc")
                for (h, lo) in ((2 * hp, 0), (2 * hp + 1, CH)):
                    base = 32 * (h % 2)
                    nc.tensor.matmul(
                        sc[:, lo:lo + CH],
                        lhsT=kh8[hp][base:base + 32, :,
                                     kt * 128:(kt + 1) * 128],
                        rhs=qh8[hp][base:base + 32, :, qcols],
                        start=True, stop=True,
                        perf_mode=DR)
                return sc

            for bi, (qc, hp) in enumerate(block_list):
                qcols = slice(qc * CH, (qc + 1) * CH)
                hA, hB = 2 * hp, 2 * hp + 1
                if hp == 0:
                    flush_mults()
                    if qc + 1 < NQC:
                        for tp in range(max((qc + 1) * TPC, NQP) // 2,
                                        (qc + 2) * TPC // 2):
                            backlog.extend(mk_qpair(tp))
                    if qc > 0:
                        for t in range((qc - 1) * TPC, qc * TPC):
                            backlog.append(mk_outproj(t))
                oaA = oa_pool.tile([65, CH], f32, tag="oa", name="oaA")
                oaB = oa_pool.tile([65, CH], f32, tag="oa", name="oaB")

                def oa_kt(kt, pT, start, stop):
                    nc.tensor.matmul(
                        oaA, lhsT=vh_aug[:, kt, hA, :], rhs=pT[:, 0:CH],
                        start=start, stop=stop)
                    nc.tensor.matmul(
                        oaB, lhsT=vh_aug[:, kt, hB, :], rhs=pT[:, CH:],
                        start=start, stop=stop)

                for kt in range(NT):
                    if kt == 0 and carried is not None:
                        pT = carried
                        carried = None
                    else:
                        sc = scores_for(qc, hp, kt)
                        pT = pT_pool.tile([128, 2 * CH], f16, tag="pT",
                                          name="pT")
                        nc.scalar.activation(pT, sc, Exp, scale=exp_scale)
                    if kt == NT - 1 and bi + 1 < len(block_list):
                        # pre-issue next block's first scores+exp: PE runs
                        # them while ACT drains exp(15); ACT never bubbles
                        # across the block boundary
                        nqc, nhp = block_list[bi + 1]
                        nsc = scores_for(nqc, nhp, 0)
                        carried = pT_pool.tile([128, 2 * CH], f16, tag="pT",
                                               name="pTc")
                        nc.scalar.activation(carried, nsc, Exp,
                                             scale=exp_scale)
                    oa_kt(kt, pT, start=(kt == 0), stop=(kt == NT - 1))
                    drain(1)
                for (h, oaT) in ((hA, oaA), (hB, oaB)):
                    dn = dn_pool.tile([1, CH], f32, tag="dn", name="dn")
                    nc.vector.reciprocal(dn, oaT[64:65, :])
                    # copy now (frees oa PSUM); normalize later
                    if qc == NQC - 1:
                        nc.scalar.copy(outT[h][:, qcols], oaT[0:64, :])
                    else:
                        nc.vector.tensor_copy(out=outT[h][:, qcols],
                                              in_=oaT[0:64, :])
                    if qc == NQC - 1:
                        # tail: skip the DRAM roundtrip; broadcast 1/den
                        # with a K=1 PE matmul into PSUM
                        dn16 = dn_pool.tile([1, CH], f16, tag="dn16",
                                            name="dn16")
                        nc.vector.tensor_copy(out=dn16, in_=dn)
                        bc = aux_pool.tile([64, CH], f32, tag="scratch",
                                           name="bc", bufs=1)
                        nc.tensor.matmul(bc, lhsT=ones_row[:, :64],
                                         rhs=dn16, start=True, stop=True)
                        nc.vector.tensor_tensor(
                            outT[h][:, qcols], outT[h][:, qcols],
                            bc, mybir.AluOpType.mult)
                    else:
                        nc.sync.dma_start(
                            out=denom_dram[h:h + 1, qcols], in_=dn)
                        rsbd = rsb_pool.tile([64, CH], f32, tag="rsbd",
                                             name="rsbd")
                        nc.sync.dma_start(
                            out=rsbd,
                            in_=denom_dram[h:h + 1, qcols].to_broadcast(
                                (64, CH)))
                        pending_mults.append(mk_mult(h, qcols, rsbd))
                drain(2)
            flush_mults()
            # tail: last q-chunk's out_proj (fins on the now-idle ACT)
            for t in range((NQC - 1) * TPC, NQC * TPC):
                backlog.append(mk_outproj(t))
            drain(len(backlog))

    return nc


def _center(Wrows):
    """Center each 64-row head group of Wrows [OF, cols]."""
    W = Wrows.reshape(HL, D, -1)
    return (W - W.mean(axis=1, keepdims=True)).reshape(OF, -1)


def _flags(inputs):
    def nz(x):
        return bool(np.any(np.asarray(x) != 0))
    return (nz(inputs['bq']), nz(inputs['bk']), nz(inputs['bv']),
            nz(inputs['bo']), nz(inputs['betaq']), nz(inputs['betak']),
            nz(inputs['betav']),
            bool(np.any(np.asarray(inputs['gq']) != 1.0)),
            bool(np.any(np.asarray(inputs['gk']) != 1.0)),
            bool(np.any(np.asarray(inputs['gv']) != 1.0)))


def _prep_core(inputs, b, hg, flags):
    (bq_nz, bk_nz, bv_nz, bo_nz, betaq_nz, betak_nz, betav_nz,
     gq_ne1, gk_ne1, gv_ne1) = flags
    q, k, v = (np.asarray(inputs['q']), np.asarray(inputs['k']),
               np.asarray(inputs['v']))
    S_ = q.shape[1]
    NT = S_ // 128
    gq, gk, gv = (np.asarray(inputs['gq'], np.float32),
                  np.asarray(inputs['gk'], np.float32),
                  np.asarray(inputs['gv'], np.float32))
    rows = slice(hg * OF, (hg + 1) * OF)

    def prep_w(W, bvec, g):
        Wc = _center(np.asarray(W, np.float32)[rows])          # [OF, E]
        bc = _center(np.asarray(bvec, np.float32)[rows, None])[:, 0]
        g_rep = np.tile(g, HL)
        Wg = Wc * g_rep[:, None]
        bg = bc * g_rep
        return (np.ascontiguousarray(Wg.T).astype(np.float16),
                bg.astype(np.float16)[None, :])

    wqT_h, bqc_h = prep_w(inputs['Wq'], inputs['bq'], gq)
    wkT_h, bkc_h = prep_w(inputs['Wk'], inputs['bk'], gk)
    wvT_h, bvc_h = prep_w(inputs['Wv'], inputs['bv'], gv)
    woT_h = np.ascontiguousarray(
        np.asarray(inputs['Wo'], np.float32)[:, rows].T).astype(np.float16)

    def mask_layout(m):
        m = np.asarray(m)[b, :, 0].astype(np.float32)
        return np.ascontiguousarray(m.reshape(NT, 128).T)

    fp8_kq = not (bq_nz or bk_nz or betaq_nz or betak_nz or gq_ne1 or gk_ne1)

    def pack8(mat, scale=1.0):
        """[E, cols] -> [kfp, p, i, cols] fp8 (d = kfp*256 + i*128 + p)."""
        import ml_dtypes
        m = (np.asarray(mat, np.float32) * scale).reshape(2, 2, 128, -1)
        return np.ascontiguousarray(
            m.transpose(0, 2, 1, 3)).astype(ml_dtypes.float8_e4m3)

    in_map = {
        "vT": np.ascontiguousarray(v[b].T).astype(np.float16),
        "wvT": wvT_h, "woT": woT_h,
        "qm": mask_layout(inputs['query_mask']),
        "km": mask_layout(inputs['key_mask']),
        "vm": mask_layout(inputs['value_mask']),
        "kvm": mask_layout(inputs['key_mask'])
        * mask_layout(inputs['value_mask']),
    }
    if fp8_kq:
        in_map["qT8"] = pack8(np.ascontiguousarray(q[b].T))
        in_map["kT8"] = pack8(np.ascontiguousarray(k[b].T))
        in_map["wq8"] = pack8(wqT_h, 16.0)
        in_map["wk8"] = pack8(wkT_h, 16.0)
    else:
        in_map["qT"] = np.ascontiguousarray(q[b].T).astype(np.float16)
        in_map["kT"] = np.ascontiguousarray(k[b].T).astype(np.float16)
        in_map["wqT"] = wqT_h
        in_map["wkT"] = wkT_h
    if bq_nz:
        in_map["bqc"] = bqc_h
    if bk_nz:
        in_map["bkc"] = bkc_h
    if bv_nz:
        in_map["bvc"] = bvc_h
    if bo_nz:
        in_map["bo2"] = (np.asarray(inputs['bo'], np.float32) / 2.0
                         ).astype(np.float16)[None, :]
    for nm, g, flag in (("gqinv2", gq, gq_ne1), ("gkinv2", gk, gk_ne1),
                        ("gvinv2", gv, gv_ne1)):
        if flag:
            in_map[nm + "_r"] = (1.0 / np.tile(g, HL) ** 2)[None, :].astype(
                np.float32)
    for nm, flag in (("betaq", betaq_nz), ("betak", betak_nz),
                     ("betav", betav_nz)):
        if flag:
            in_map[nm + "_r"] = np.tile(
                np.asarray(inputs[nm], np.float32), HL)[None, :]
    return in_map


def kernel(**inputs):
    flags = _flags(inputs)
    key = (S, flags)
    if key not in _prog_cache:
        nc = build_program(S, flags)
        if not nc.is_finalized():
            nc.finalize()
        _prog_cache[key] = nc
    nc = _prog_cache[key]

    in_maps = [_prep_core(inputs, c // 2, c % 2, flags) for c in range(8)]
    res = run_bass_kernel_spmd(nc, in_maps, core_ids=list(range(8)))
    out = np.zeros((B, S, E), np.float32)
    for c in range(8):
        out[c // 2] += np.asarray(res.results[c]["out"], np.float32)
    return out
